# revision 4
# baseline (speedup 1.0000x reference)
"""Beam-search top-k (mask pad + add beam scores + top-16 over beam*vocab) on 8 trn2 cores.

Sharding: batch dim (64 rows) split across 8 cores, 8 rows/core, no cross-core comm.

Per-core device pipeline (Bass/Tile):
  tile [128, 25136] f32, partition p = t*16 + h*8 + b  (t=batch row, b=beam, h=vocab half)
     h=0 holds vocab [0, 25136); h=1 holds vocab [25136, 50257) + 15 -inf pads
  1. 16 DMAs in, -inf tail memset, pad-token col masked via per-partition add
  2. segmented reduce_max over groups of 8 -> M [128, 3142] (+2 -inf pad cols)
  3. M += beam score (per-partition scalar); gpsimd topk#1 over each token's
     16 partitions (n = 16*3144 = 50304) -> top-256 groups, sorted, with indices
  4. top-16 group cols -> ap_gather of those cols across the token's 16
     partitions (16 cols x 16 parts x 8 elems superset) -> + score -> topk#2
  5. host decodes the 256 sorted candidates per row, dedups (a col can win for
     more than one partition; 256 >= 16 * max multiplicity 16 covers worst case),
     takes top-16.
"""

import sys

sys.path.insert(0, "/opt/trn_rl_repo")

import numpy as np

BSZ, BEAM, VOCAB, VK = 64, 8, 50257, 16
NCORES = 8
ROWS = BSZ // NCORES   # 8 tokens (batch rows) per core
F = 25136              # per-partition elems
H1 = VOCAB - F         # 25121 valid in h=1 partitions
P = 128
G8 = 8                 # reduce group size
NG = F // G8           # 3142 groups per partition
NGP = 3144             # padded group cols (flat n must be % 128)
NTOPK = 16 * NGP       # 50304 flat per token
NEG = float("-inf")

_CACHE = {}


def _emit_topk(nc, bass_isa, out_ap, in_ap, tokens, n, k=256):
    g = nc.gpsimd
    return g.add_instruction(
        bass_isa.InstTopk(
            name=f"I-{nc.next_id()}",
            ins=[g.lower_ap(in_ap, for_isa=True)],
            outs=[g.lower_ap(out_ap, for_isa=True)],
            _tokens=tokens,
            _n=n,
            _k=k,
        )
    )


def _emit_ap_gather(nc, bass_isa, out_ap, in_ap, idxs_ap, channels, num_elems, d, num_idxs):
    g = nc.gpsimd
    return g.add_instruction(
        bass_isa.InstAPGather(
            name=f"I-{nc.next_id()}",
            ins=[g.lower_ap(in_ap, for_isa=True), g.lower_ap(idxs_ap, for_isa=True)],
            outs=[g.lower_ap(out_ap, for_isa=True)],
            _channels=channels,
            _num_elems=num_elems,
            _d=d,
            _num_idxs=num_idxs,
        )
    )


def _build():
    import concourse.bacc as bacc
    import concourse.mybir as mybir
    from concourse import bass_isa
    from concourse.tile import TileContext

    nc = bacc.Bacc("TRN2", target_bir_lowering=False, debug=False)
    x = nc.dram_tensor("x", [ROWS, BEAM, VOCAB], mybir.dt.float32, kind="ExternalInput").ap()
    scol = nc.dram_tensor("scol", [P, 1], mybir.dt.float32, kind="ExternalInput").ap()
    mcol = nc.dram_tensor("mcol", [P, 1], mybir.dt.float32, kind="ExternalInput").ap()
    out1 = nc.dram_tensor("out1", [ROWS, 32], mybir.dt.uint32, kind="ExternalOutput").ap()
    out2 = nc.dram_tensor("out2", [P, 32], mybir.dt.uint32, kind="ExternalOutput").ap()

    with TileContext(nc) as tc:
        with tc.tile_pool(name="main", bufs=1) as pool:
            tile = pool.tile([P, F], mybir.dt.float32)
            M = pool.tile([P, NGP], mybir.dt.float32)
            Gbuf = pool.tile([P, NGP], mybir.dt.float32)
            sc = pool.tile([P, 1], mybir.dt.float32)
            mc = pool.tile([P, 1], mybir.dt.float32)
            W = pool.tile([P, 1], mybir.dt.uint32)
            Wf = pool.tile([P, 1], mybir.dt.float32)
            Wg = pool.tile([P, 1], mybir.dt.float32)
            Wi = pool.tile([P, 1], mybir.dt.int16)
            T1 = pool.tile([P, 32], mybir.dt.uint32)
            T2 = pool.tile([P, 32], mybir.dt.uint32)

            nc.sync.dma_start(out=sc[:, :], in_=scol)
            nc.sync.dma_start(out=mc[:, :], in_=mcol)

            # tail pad; the h=0 DMAs overwrite their partitions' copy afterwards
            nc.vector.memset(tile[:, H1:F], NEG)

            for t in range(ROWS):
                for h in range(2):
                    ln = F if h == 0 else H1
                    p0 = t * 16 + h * 8
                    nc.sync.dma_start(
                        out=tile[p0:p0 + 8, 0:ln],
                        in_=x[t:t + 1, :, h * F:h * F + ln],
                    )

            # pad-token (vocab id 1) mask: -inf on h=0 partitions via mcol
            nc.vector.tensor_scalar_add(tile[:, 1:2], tile[:, 1:2], mc[:, 0:1])

            # group maxima
            t3 = tile[:, :].rearrange("p (g w) -> p g w", w=G8)
            nc.vector.reduce_max(out=M[:, 0:NG], in_=t3, axis=mybir.AxisListType.X)
            nc.vector.memset(M[:, NG:NGP], NEG)
            nc.vector.tensor_scalar_add(M[:, 0:NG], M[:, 0:NG], sc[:, 0:1])

            _emit_topk(nc, bass_isa, T1[:, :], M[:, :], tokens=ROWS, n=NTOPK)

            # winner cols: ranks 240+j of token t at (partition t*16+15, col 16+j)
            for t in range(ROWS):
                nc.sync.dma_start(
                    out=W[t * 16:(t + 1) * 16, 0:1],
                    in_=T1[t * 16 + 15:t * 16 + 16, 16:32],
                )
            # c = g % NGP via conditional-subtract ladder in f32 (g < 50304, exact)
            nc.vector.tensor_copy(out=Wf[:, :], in_=W[:, :])
            for mul in (8, 4, 2, 1):
                kq = float(mul * NGP)
                nc.vector.tensor_scalar(
                    out=Wg[:, :], in0=Wf[:, :], scalar1=kq, scalar2=None,
                    op0=mybir.AluOpType.is_ge,
                )
                nc.vector.scalar_tensor_tensor(
                    out=Wf[:, :], in0=Wg[:, :], scalar=-kq, in1=Wf[:, :],
                    op0=mybir.AluOpType.mult, op1=mybir.AluOpType.add,
                )
            nc.vector.tensor_copy(out=Wi[:, :], in_=Wf[:, :])

            # gather the winning cols' raw groups across each token's 16 partitions
            nc.vector.memset(Gbuf[:, 128:NGP], NEG)
            _emit_ap_gather(
                nc, bass_isa, Gbuf[:, 0:128], tile[:, :], Wi[:, :],
                channels=P, num_elems=NG, d=G8, num_idxs=16,
            )
            nc.vector.tensor_scalar_add(Gbuf[:, 0:128], Gbuf[:, 0:128], sc[:, 0:1])

            _emit_topk(nc, bass_isa, T2[:, :], Gbuf[:, :], tokens=ROWS, n=NTOPK)

            for t in range(ROWS):
                nc.sync.dma_start(out=out1[t:t + 1, :], in_=T1[t * 16 + 15:t * 16 + 16, :])
            nc.sync.dma_start(out=out2, in_=T2[:, :])

    nc.compile()
    return nc


def _get_nc():
    if "nc" not in _CACHE:
        _CACHE["nc"] = _build()
    return _CACHE["nc"]


def _side_inputs(scores_shard: np.ndarray, step: int):
    scol = np.zeros((P, 1), np.float32)
    mcol = np.zeros((P, 1), np.float32)
    for t in range(ROWS):
        for b in range(BEAM):
            for h in range(2):
                p = t * 16 + h * 8 + b
                if step == 0:
                    scol[p, 0] = 0.0 if b == 0 else NEG
                else:
                    scol[p, 0] = scores_shard[t, b]
                if h == 0:
                    mcol[p, 0] = NEG
    return scol, mcol


def _decode(out1_u32: np.ndarray, out2_u32: np.ndarray, step: int):
    vals = np.zeros((ROWS, VK), np.float32)
    vocab = np.zeros((ROWS, VK), np.int32)
    beams = np.zeros((ROWS, VK), np.int32)
    for t in range(ROWS):
        cj = out1_u32[t, 16:32].astype(np.int64) % NGP
        block = out2_u32[t * 16:(t + 1) * 16, :]
        vals256 = block[:, :16].view(np.float32).reshape(-1)   # ranks 0..255 asc
        idx256 = block[:, 16:32].reshape(-1).astype(np.int64)
        seen = set()
        cand = []  # (val, beam, vocab)
        for r in range(255, -1, -1):
            g2 = idx256[r]
            q2, colg = divmod(g2, NGP)
            j, e = divmod(colg, G8)
            h, b = divmod(q2, 8)
            v = h * F + cj[j] * G8 + e
            key = (int(b), int(v))
            if key in seen:
                continue
            seen.add(key)
            cand.append((vals256[r], int(b), int(v)))
            if len(cand) == VK + 16:
                break
        assert len(cand) >= VK, f"only {len(cand)} unique candidates for row {t}"
        # match jax.lax.top_k tie-break: equal values ordered by ascending flat idx
        cand.sort(key=lambda c: (-c[0], c[1] * VOCAB + c[2]))
        for k in range(VK):
            vals[t, k] = cand[k][0]
            vocab[t, k] = cand[k][2]
            beams[t, k] = 0 if step == 0 else cand[k][1]
    return vals, vocab, beams


def _run(lprobs: np.ndarray, scores: np.ndarray, step: int, trace: bool = False):
    from concourse.bass_utils import run_bass_kernel_spmd

    nc = _get_nc()
    in_maps = []
    for c in range(NCORES):
        shard = np.ascontiguousarray(lprobs[c * ROWS:(c + 1) * ROWS])
        scol, mcol = _side_inputs(scores[c * ROWS:(c + 1) * ROWS], step)
        in_maps.append({"x": shard, "scol": scol, "mcol": mcol})
    res = run_bass_kernel_spmd(nc, in_maps, core_ids=list(range(NCORES)), trace=trace)
    return res


def kernel(lprobs, scores, step):
    lprobs = np.asarray(lprobs, dtype=np.float32)
    scores = np.asarray(scores, dtype=np.float32)
    step = int(step)

    res = _run(lprobs, scores, step)

    scores_buf = np.zeros((BSZ, VK), np.float32)
    indices_buf = np.zeros((BSZ, VK), np.int32)
    beams_buf = np.zeros((BSZ, VK), np.int32)
    for c in range(NCORES):
        o = res.results[c]
        v, vi, bi = _decode(o["out1"], o["out2"], step)
        rows = slice(c * ROWS, (c + 1) * ROWS)
        scores_buf[rows] = v
        indices_buf[rows] = vi
        beams_buf[rows] = bi
    return scores_buf, indices_buf, beams_buf


# revision 11
# speedup vs baseline: 15.9883x; 15.9883x over previous
"""Beam-search top-k (mask pad + add beam scores + top-16 over beam*vocab) on 8 trn2 cores.

Sharding: batch dim (64 rows) split across 8 cores, 8 rows/core, no cross-core comm.

Per-core device pipeline (Bass/Tile):
  tile [128, 25136] f32, partition p = t*16 + h*8 + b  (t=batch row, b=beam, h=vocab half)
     h=0 holds vocab [0, 25136); h=1 holds vocab [25136, 50257) + 15 -inf pads
  1. 16 DMAs in, -inf tail memset, pad-token col masked via per-partition add
  2. segmented reduce_max over groups of 8 -> M [128, 3142] (+2 -inf pad cols)
  3. M += beam score (per-partition scalar); gpsimd topk#1 over each token's
     16 partitions (n = 16*3144 = 50304) -> top-256 groups, sorted, with indices
  4. top-16 group cols -> ap_gather of those cols across the token's 16
     partitions (16 cols x 16 parts x 8 elems superset) -> + score -> topk#2
  5. host decodes the 256 sorted candidates per row, dedups (a col can win for
     more than one partition; 256 >= 16 * max multiplicity 16 covers worst case),
     takes top-16.
"""

import sys

sys.path.insert(0, "/opt/trn_rl_repo")

import numpy as np

BSZ, BEAM, VOCAB, VK = 64, 8, 50257, 16
NCORES = 8
ROWS = BSZ // NCORES   # 8 tokens (batch rows) per core
F = 25136              # per-partition elems
CH0 = VOCAB - F        # 25121: h=1 partitions cover vocab [25121, 50257)
                       # (15-element overlap with h=0; host dedup removes dups)
P = 128
G8 = 8                 # reduce group size
NG = F // G8           # 3142 groups per partition
NGP = 3144             # padded group cols (flat n must be % 128)
NTOPK = 16 * NGP       # 50304 flat per token
NEG = float("-inf")

_CACHE = {}


def _emit_topk(nc, bass_isa, out_ap, in_ap, tokens, n, k=256):
    g = nc.gpsimd
    return g.add_instruction(
        bass_isa.InstTopk(
            name=f"I-{nc.next_id()}",
            ins=[g.lower_ap(in_ap, for_isa=True)],
            outs=[g.lower_ap(out_ap, for_isa=True)],
            _tokens=tokens,
            _n=n,
            _k=k,
        )
    )


def _emit_ap_gather(nc, bass_isa, out_ap, in_ap, idxs_ap, channels, num_elems, d, num_idxs):
    g = nc.gpsimd
    return g.add_instruction(
        bass_isa.InstAPGather(
            name=f"I-{nc.next_id()}",
            ins=[g.lower_ap(in_ap, for_isa=True), g.lower_ap(idxs_ap, for_isa=True)],
            outs=[g.lower_ap(out_ap, for_isa=True)],
            _channels=channels,
            _num_elems=num_elems,
            _d=d,
            _num_idxs=num_idxs,
        )
    )


def _build():
    import concourse.bacc as bacc
    import concourse.mybir as mybir
    from concourse import bass_isa
    from concourse.bass_types import AP
    from concourse.tile import TileContext

    nc = bacc.Bacc("TRN2", target_bir_lowering=False, debug=False)
    x = nc.dram_tensor("x", [ROWS, BEAM, VOCAB], mybir.dt.float32, kind="ExternalInput").ap()
    scol = nc.dram_tensor("scol", [P, 1], mybir.dt.float32, kind="ExternalInput").ap()
    mcol = nc.dram_tensor("mcol", [P, 1], mybir.dt.float32, kind="ExternalInput").ap()
    out1 = nc.dram_tensor("out1", [ROWS, 32], mybir.dt.uint32, kind="ExternalOutput").ap()
    out2 = nc.dram_tensor("out2", [P, 32], mybir.dt.uint32, kind="ExternalOutput").ap()

    with TileContext(nc) as tc:
        with tc.tile_pool(name="main", bufs=1) as pool:
            tile = pool.tile([P, F], mybir.dt.float32)
            M = pool.tile([P, NGP], mybir.dt.float32)
            Gbuf = pool.tile([P, NGP], mybir.dt.float32)
            sc = pool.tile([P, 1], mybir.dt.float32)
            mc = pool.tile([P, 1], mybir.dt.float32)
            W = pool.tile([P, 1], mybir.dt.uint32)
            Wf = pool.tile([P, 1], mybir.dt.float32)
            Wg = pool.tile([P, 1], mybir.dt.float32)
            Wi = pool.tile([P, 1], mybir.dt.int16)
            T1 = pool.tile([P, 32], mybir.dt.uint32)
            T2 = pool.tile([P, 32], mybir.dt.uint32)

            nc.sync.dma_start(out=sc[:, :], in_=scol)
            nc.sync.dma_start(out=mc[:, :], in_=mcol)

            # chunked loads, all 128 partitions per DMA. Partition p = (t*8+b)*2+h
            # reads x_flat[(t*8+b)*50257 + h*25121 + col] -- the (t,b) dims merge
            # into one stride-50257 dim so the src AP is 3-D and balances against
            # the canonical [128, ln] dst. Per-chunk reduce pipelines with the
            # next chunk's DMA.
            chunks = [(i * 1576, 1576) for i in range(15)] + [(15 * 1576, 1496)]
            first = True
            for o, ln in chunks:
                src = AP(
                    tensor=x.tensor, offset=o,
                    ap=[[VOCAB, ROWS * BEAM], [CH0, 2], [1, ln]],
                )
                nc.sync.dma_start(out=tile[:, o:o + ln], in_=src)
                if first:
                    # pad-token (vocab id 1) mask: -inf on h=0 partitions via mcol
                    nc.vector.tensor_scalar_add(tile[:, 1:2], tile[:, 1:2], mc[:, 0:1])
                    first = False
                t3 = tile[:, o:o + ln].rearrange("p (g w) -> p g w", w=G8)
                nc.vector.reduce_max(
                    out=M[:, o // G8:(o + ln) // G8], in_=t3, axis=mybir.AxisListType.X
                )

            nc.vector.memset(M[:, NG:NGP], NEG)
            nc.vector.tensor_scalar_add(M[:, 0:NG], M[:, 0:NG], sc[:, 0:1])

            _emit_topk(nc, bass_isa, T1[:, :], M[:, :], tokens=ROWS, n=NTOPK)

            # winner cols: ranks 240+j of token t at (partition t*16+15, col 16+j)
            for t in range(ROWS):
                nc.sync.dma_start(
                    out=W[t * 16:(t + 1) * 16, 0:1],
                    in_=T1[t * 16 + 15:t * 16 + 16, 16:32],
                )
            # c = g % NGP via conditional-subtract ladder in f32 (g < 50304, exact)
            nc.vector.tensor_copy(out=Wf[:, :], in_=W[:, :])
            for mul in (8, 4, 2, 1):
                kq = float(mul * NGP)
                nc.vector.tensor_scalar(
                    out=Wg[:, :], in0=Wf[:, :], scalar1=kq, scalar2=None,
                    op0=mybir.AluOpType.is_ge,
                )
                nc.vector.scalar_tensor_tensor(
                    out=Wf[:, :], in0=Wg[:, :], scalar=-kq, in1=Wf[:, :],
                    op0=mybir.AluOpType.mult, op1=mybir.AluOpType.add,
                )
            nc.vector.tensor_copy(out=Wi[:, :], in_=Wf[:, :])

            # gather the winning cols' raw groups across each token's 16 partitions
            nc.vector.memset(Gbuf[:, 128:NGP], NEG)
            _emit_ap_gather(
                nc, bass_isa, Gbuf[:, 0:128], tile[:, :], Wi[:, :],
                channels=P, num_elems=NG, d=G8, num_idxs=16,
            )
            nc.vector.tensor_scalar_add(Gbuf[:, 0:128], Gbuf[:, 0:128], sc[:, 0:1])

            _emit_topk(nc, bass_isa, T2[:, :], Gbuf[:, :], tokens=ROWS, n=NTOPK)

            for t in range(ROWS):
                nc.sync.dma_start(out=out1[t:t + 1, :], in_=T1[t * 16 + 15:t * 16 + 16, :])
            nc.sync.dma_start(out=out2, in_=T2[:, :])

    nc.compile()
    return nc


def _get_nc():
    if "nc" not in _CACHE:
        _CACHE["nc"] = _build()
    return _CACHE["nc"]


def _side_inputs(scores_shard: np.ndarray, step: int):
    scol = np.zeros((P, 1), np.float32)
    mcol = np.zeros((P, 1), np.float32)
    for t in range(ROWS):
        for b in range(BEAM):
            for h in range(2):
                p = t * 16 + b * 2 + h
                if step == 0:
                    scol[p, 0] = 0.0 if b == 0 else NEG
                else:
                    scol[p, 0] = scores_shard[t, b]
                if h == 0:
                    mcol[p, 0] = NEG
    return scol, mcol


def _decode(out1_u32: np.ndarray, out2_u32: np.ndarray, step: int):
    vals = np.zeros((ROWS, VK), np.float32)
    vocab = np.zeros((ROWS, VK), np.int32)
    beams = np.zeros((ROWS, VK), np.int32)
    for t in range(ROWS):
        cj = out1_u32[t, 16:32].astype(np.int64) % NGP
        block = out2_u32[t * 16:(t + 1) * 16, :]
        vals256 = block[:, :16].view(np.float32).reshape(-1)   # ranks 0..255 asc
        idx256 = block[:, 16:32].reshape(-1).astype(np.int64)
        seen = set()
        cand = []  # (val, beam, vocab)
        for r in range(255, -1, -1):
            g2 = idx256[r]
            q2, colg = divmod(g2, NGP)
            j, e = divmod(colg, G8)
            b, h = divmod(q2, 2)
            v = h * CH0 + cj[j] * G8 + e
            key = (int(b), int(v))
            if key in seen:
                continue
            seen.add(key)
            cand.append((vals256[r], int(b), int(v)))
            if len(cand) == VK + 16:
                break
        assert len(cand) >= VK, f"only {len(cand)} unique candidates for row {t}"
        # match jax.lax.top_k tie-break: equal values ordered by ascending flat idx
        cand.sort(key=lambda c: (-c[0], c[1] * VOCAB + c[2]))
        for k in range(VK):
            vals[t, k] = cand[k][0]
            vocab[t, k] = cand[k][2]
            beams[t, k] = 0 if step == 0 else cand[k][1]
    return vals, vocab, beams


def _run(lprobs: np.ndarray, scores: np.ndarray, step: int, trace: bool = False):
    from concourse.bass_utils import run_bass_kernel_spmd

    nc = _get_nc()
    in_maps = []
    for c in range(NCORES):
        shard = np.ascontiguousarray(lprobs[c * ROWS:(c + 1) * ROWS])
        scol, mcol = _side_inputs(scores[c * ROWS:(c + 1) * ROWS], step)
        in_maps.append({"x": shard, "scol": scol, "mcol": mcol})
    res = run_bass_kernel_spmd(nc, in_maps, core_ids=list(range(NCORES)), trace=trace)
    return res


def kernel(lprobs, scores, step):
    lprobs = np.asarray(lprobs, dtype=np.float32)
    scores = np.asarray(scores, dtype=np.float32)
    step = int(step)

    res = _run(lprobs, scores, step)

    scores_buf = np.zeros((BSZ, VK), np.float32)
    indices_buf = np.zeros((BSZ, VK), np.int32)
    beams_buf = np.zeros((BSZ, VK), np.int32)
    for c in range(NCORES):
        o = res.results[c]
        v, vi, bi = _decode(o["out1"], o["out2"], step)
        rows = slice(c * ROWS, (c + 1) * ROWS)
        scores_buf[rows] = v
        indices_buf[rows] = vi
        beams_buf[rows] = bi
    return scores_buf, indices_buf, beams_buf


# revision 15
# speedup vs baseline: 18.0340x; 1.1279x over previous
"""Beam-search top-k (mask pad + add beam scores + top-16 over beam*vocab) on 8 trn2 cores.

Sharding: batch dim (64 rows) split across 8 cores, 8 rows/core, no cross-core comm.

Per-core device pipeline (Bass/Tile):
  tile [128, 25136] f32, partition p = (t*8+b)*2 + h  (t=batch row, b=beam, h=half)
     h=0 holds vocab [0, 25136); h=1 holds vocab [25121, 50257)
     (15-element overlap instead of padding; host dedup removes duplicates)
  1. gpsimd topk ucode library loaded up front (overlaps the DMA phase)
  2. 16 chunked DMAs, each all-128-partitions (the (t,b) dims merge into one
     stride-50257 dim, so src APs stay 3-D); per-chunk segmented reduce_max
     over groups of 8 -> M [128, 3142] (+2 -inf pad cols) pipelines with DMA
  3. M += beam score; gpsimd topk#1 over each token's 16 partitions
     (n = 16*3144 = 50304) -> top-256 groups, sorted, with flat indices
  4. winners j=0..15 (ranks 240+j): decompose g = q*3144+c with an is_ge
     ladder (f32-exact), rebuild the DRAM element offset of group (q,c), and
     indirect-DMA-gather each winner's 8 contiguous elements into partition
     t*16+j, plus its beam score; bias; topk#2 over the -inf-padded buffer
  5. host decodes the 256 sorted candidates per row, dedups (a group can win
     for more than one rank; 256 >= 16 * max multiplicity covers worst case),
     sorts ties by flat index like jax.lax.top_k, takes top-16.
"""

import sys

sys.path.insert(0, "/opt/trn_rl_repo")

import numpy as np

BSZ, BEAM, VOCAB, VK = 64, 8, 50257, 16
NCORES = 8
ROWS = BSZ // NCORES   # 8 tokens (batch rows) per core
F = 25136              # per-partition elems
CH0 = VOCAB - F        # 25121: h=1 partitions cover vocab [25121, 50257)
P = 128
G8 = 8                 # reduce group size
NG = F // G8           # 3142 groups per partition
NGP = 3144             # padded group cols (flat n must be % 128, > 50000)
NTOPK = 16 * NGP       # 50304 flat per token
NEL = ROWS * BEAM * VOCAB  # 3216448 elements in the per-core shard
NEG = float("-inf")

_CACHE = {}


def _emit_topk(nc, bass_isa, out_ap, in_ap, tokens, n, k=256):
    g = nc.gpsimd
    return g.add_instruction(
        bass_isa.InstTopk(
            name=f"I-{nc.next_id()}",
            ins=[g.lower_ap(in_ap, for_isa=True)],
            outs=[g.lower_ap(out_ap, for_isa=True)],
            _tokens=tokens,
            _n=n,
            _k=k,
        )
    )


def _build():
    import concourse.bacc as bacc
    import concourse.bass as bass
    import concourse.mybir as mybir
    from concourse import bass_isa, library_config
    from concourse.bass_types import AP
    from concourse.tile import TileContext

    ALU = mybir.AluOpType

    nc = bacc.Bacc("TRN2", target_bir_lowering=False, debug=False)
    x = nc.dram_tensor("x", [ROWS, BEAM, VOCAB], mybir.dt.float32, kind="ExternalInput").ap()
    scol = nc.dram_tensor("scol", [P, 1], mybir.dt.float32, kind="ExternalInput").ap()
    mcol = nc.dram_tensor("mcol", [P, 1], mybir.dt.float32, kind="ExternalInput").ap()
    tcol = nc.dram_tensor("tcol", [P, 1], mybir.dt.float32, kind="ExternalInput").ap()
    t8col = nc.dram_tensor("t8col", [P, 1], mybir.dt.float32, kind="ExternalInput").ap()
    svec = nc.dram_tensor("svec", [ROWS * BEAM, 1], mybir.dt.float32, kind="ExternalInput").ap()
    out1 = nc.dram_tensor("out1", [ROWS, 32], mybir.dt.uint32, kind="ExternalOutput").ap()
    out2 = nc.dram_tensor("out2", [P, 32], mybir.dt.uint32, kind="ExternalOutput").ap()

    with TileContext(nc) as tc:
        with tc.tile_pool(name="main", bufs=1) as pool:
            tile = pool.tile([P, F], mybir.dt.float32)
            M = pool.tile([P, NGP], mybir.dt.float32)
            Gbuf = pool.tile([P, NGP], mybir.dt.float32)
            sc = pool.tile([P, 1], mybir.dt.float32)
            mc = pool.tile([P, 1], mybir.dt.float32)
            tcl = pool.tile([P, 1], mybir.dt.float32)
            t8c = pool.tile([P, 1], mybir.dt.float32)
            W = pool.tile([P, 1], mybir.dt.uint32)
            Wf = pool.tile([P, 1], mybir.dt.float32)
            Sg = pool.tile([P, 1], mybir.dt.float32)
            bits = [pool.tile([P, 1], mybir.dt.float32, name=f"bit{i}", tag=f"bit{i}") for i in range(4)]
            Qf = pool.tile([P, 1], mybir.dt.float32)
            Bf = pool.tile([P, 1], mybir.dt.float32)
            Ef = pool.tile([P, 1], mybir.dt.float32)
            Eu = pool.tile([P, 1], mybir.dt.uint32)
            Su = pool.tile([P, 1], mybir.dt.uint32)
            T1 = pool.tile([P, 32], mybir.dt.uint32)
            T2 = pool.tile([P, 32], mybir.dt.uint32)

            # topk ucode: load once, early -- overlaps the DMA/reduce front and
            # avoids any mid-kernel library switch (the gathers are plain DMAs).
            nc.gpsimd.load_library(library_config.topk)

            nc.sync.dma_start(out=sc[:, :], in_=scol)
            nc.sync.dma_start(out=mc[:, :], in_=mcol)
            nc.sync.dma_start(out=tcl[:, :], in_=tcol)
            nc.sync.dma_start(out=t8c[:, :], in_=t8col)

            # candidate buffer pad: set once, off the critical path
            nc.vector.memset(Gbuf[:, G8:NGP], NEG)

            # chunked loads, all 128 partitions per DMA; per-chunk reduce
            chunks = [(i * 1576, 1576) for i in range(15)] + [(15 * 1576, 1496)]
            first = True
            for o, ln in chunks:
                src = AP(
                    tensor=x.tensor, offset=o,
                    ap=[[VOCAB, ROWS * BEAM], [CH0, 2], [1, ln]],
                )
                nc.sync.dma_start(out=tile[:, o:o + ln], in_=src)
                if first:
                    # pad-token (vocab id 1) mask: -inf on h=0 partitions
                    nc.vector.tensor_scalar_add(tile[:, 1:2], tile[:, 1:2], mc[:, 0:1])
                    first = False
                t3 = tile[:, o:o + ln].rearrange("p (g w) -> p g w", w=G8)
                nc.vector.reduce_max(
                    out=M[:, o // G8:(o + ln) // G8], in_=t3, axis=mybir.AxisListType.X
                )

            nc.vector.memset(M[:, NG:NGP], NEG)
            nc.vector.tensor_scalar_add(M[:, 0:NG], M[:, 0:NG], sc[:, 0:1])

            _emit_topk(nc, bass_isa, T1[:, :], M[:, :], tokens=ROWS, n=NTOPK)

            # winners: ranks 240+j of token t at (partition t*16+15, col 16+j);
            # scatter index j to partition t*16+j
            for t in range(ROWS):
                nc.sync.dma_start(
                    out=W[t * 16:(t + 1) * 16, 0:1],
                    in_=T1[t * 16 + 15:t * 16 + 16, 16:32],
                )

            # decompose g = q*3144 + c exactly in f32 (g < 50304 < 2^24):
            # conditional-subtract ladder, keeping the quotient bits
            nc.vector.tensor_copy(out=Wf[:, :], in_=W[:, :])
            for i, mul in enumerate((8, 4, 2, 1)):
                kq = float(mul * NGP)
                nc.vector.tensor_scalar(
                    out=bits[i][:, :], in0=Wf[:, :], scalar1=kq, scalar2=None,
                    op0=ALU.is_ge,
                )
                nc.vector.scalar_tensor_tensor(
                    out=Wf[:, :], in0=bits[i][:, :], scalar=-kq, in1=Wf[:, :],
                    op0=ALU.mult, op1=ALU.add,
                )
            # Wf now holds c; q = 8*s8+4*s4+2*s2+s1, b = q//2 = 4*s8+2*s4+s2
            nc.vector.scalar_tensor_tensor(
                out=Qf[:, :], in0=bits[0][:, :], scalar=2.0, in1=bits[1][:, :],
                op0=ALU.mult, op1=ALU.add,
            )
            nc.vector.scalar_tensor_tensor(
                out=Bf[:, :], in0=Qf[:, :], scalar=2.0, in1=bits[2][:, :],
                op0=ALU.mult, op1=ALU.add,
            )  # Bf = 4*s8+2*s4+s2 = b
            nc.vector.scalar_tensor_tensor(
                out=Qf[:, :], in0=Bf[:, :], scalar=2.0, in1=bits[3][:, :],
                op0=ALU.mult, op1=ALU.add,
            )  # Qf = q
            # DRAM element offset of group (t, q, c):
            #   E = t*402056 + q*25121 + b*15 + c*8   (all f32-exact, < 2^24)
            nc.vector.scalar_tensor_tensor(
                out=Ef[:, :], in0=Qf[:, :], scalar=float(CH0), in1=tcl[:, :],
                op0=ALU.mult, op1=ALU.add,
            )
            nc.vector.scalar_tensor_tensor(
                out=Ef[:, :], in0=Bf[:, :], scalar=15.0, in1=Ef[:, :],
                op0=ALU.mult, op1=ALU.add,
            )
            nc.vector.scalar_tensor_tensor(
                out=Ef[:, :], in0=Wf[:, :], scalar=float(G8), in1=Ef[:, :],
                op0=ALU.mult, op1=ALU.add,
            )
            nc.vector.tensor_copy(out=Eu[:, :], in_=Ef[:, :])
            # score offset: t*8 + b
            nc.vector.tensor_tensor(
                out=Bf[:, :], in0=Bf[:, :], in1=t8c[:, :], op=ALU.add,
            )
            nc.vector.tensor_copy(out=Su[:, :], in_=Bf[:, :])

            # gather each winner's 8 contiguous elements + its beam score
            xin = AP(tensor=x.tensor, offset=0, ap=[[1, NEL], [1, 1]])
            nc.gpsimd.indirect_dma_start(
                out=Gbuf[:, 0:G8], out_offset=None, in_=xin,
                in_offset=bass.IndirectOffsetOnAxis(ap=Eu[:, :], axis=0),
            )
            sin = AP(tensor=svec.tensor, offset=0, ap=[[1, ROWS * BEAM], [1, 1]])
            nc.gpsimd.indirect_dma_start(
                out=Sg[:, :], out_offset=None, in_=sin,
                in_offset=bass.IndirectOffsetOnAxis(ap=Su[:, :], axis=0),
            )
            nc.vector.tensor_scalar_add(Gbuf[:, 0:G8], Gbuf[:, 0:G8], Sg[:, 0:1])

            # pad-token fix: the gather reads raw DRAM, so a winning group
            # (q even, c == 0) carries the unmasked vocab-1 value at e == 1.
            # Push it out of competition with a huge negative bias.
            nc.vector.tensor_scalar(
                out=Ef[:, :], in0=Wf[:, :], scalar1=0.0, scalar2=None,
                op0=ALU.is_le,
            )  # c == 0  (c >= 0 always)
            nc.vector.tensor_scalar(
                out=Bf[:, :], in0=bits[3][:, :], scalar1=-1.0, scalar2=1.0,
                op0=ALU.mult, op1=ALU.add,
            )  # 1 - s1 = (h == 0)
            nc.vector.tensor_tensor(
                out=Ef[:, :], in0=Ef[:, :], in1=Bf[:, :], op=ALU.mult,
            )
            nc.vector.scalar_tensor_tensor(
                out=Gbuf[:, 1:2], in0=Ef[:, :], scalar=-1e38, in1=Gbuf[:, 1:2],
                op0=ALU.mult, op1=ALU.add,
            )

            _emit_topk(nc, bass_isa, T2[:, :], Gbuf[:, :], tokens=ROWS, n=NTOPK)

            for t in range(ROWS):
                nc.sync.dma_start(out=out1[t:t + 1, :], in_=T1[t * 16 + 15:t * 16 + 16, :])
            nc.sync.dma_start(out=out2, in_=T2[:, :])

    nc.compile()
    return nc


def _get_nc():
    if "nc" not in _CACHE:
        _CACHE["nc"] = _build()
    return _CACHE["nc"]


def _side_inputs(scores_shard: np.ndarray, step: int):
    scol = np.zeros((P, 1), np.float32)
    mcol = np.zeros((P, 1), np.float32)
    tcol = np.zeros((P, 1), np.float32)
    t8col = np.zeros((P, 1), np.float32)
    svec = np.zeros((ROWS * BEAM, 1), np.float32)
    for t in range(ROWS):
        for b in range(BEAM):
            sv = (0.0 if b == 0 else NEG) if step == 0 else float(scores_shard[t, b])
            svec[t * BEAM + b, 0] = sv
            for h in range(2):
                p = t * 16 + b * 2 + h
                scol[p, 0] = sv
                if h == 0:
                    mcol[p, 0] = NEG
    for p in range(P):
        t = p // 16
        tcol[p, 0] = float(t * BEAM * VOCAB)
        t8col[p, 0] = float(t * BEAM)
    return scol, mcol, tcol, t8col, svec


def _decode(out1_u32: np.ndarray, out2_u32: np.ndarray, step: int):
    vals = np.zeros((ROWS, VK), np.float32)
    vocab = np.zeros((ROWS, VK), np.int32)
    beams = np.zeros((ROWS, VK), np.int32)
    for t in range(ROWS):
        g1 = out1_u32[t, 16:32].astype(np.int64)      # winner flat idx, j=0..15
        cj = g1 % NGP
        qj = g1 // NGP
        block = out2_u32[t * 16:(t + 1) * 16, :]
        vals256 = block[:, :16].view(np.float32).reshape(-1)   # ranks 0..255 asc
        idx256 = block[:, 16:32].reshape(-1).astype(np.int64)
        seen = set()
        cand = []  # (val, beam, vocab)
        for r in range(255, -1, -1):
            if not np.isfinite(vals256[r]) or vals256[r] < -1e37:
                break  # into the -inf padding / masked entries; no more candidates
            g2 = idx256[r]
            j, e = divmod(g2, NGP)
            b, h = divmod(int(qj[j]), 2)
            v = h * CH0 + int(cj[j]) * G8 + int(e)
            if v == 1:
                continue  # pad token (belt & braces; device already demotes it)
            key = (b, v)
            if key in seen:
                continue
            seen.add(key)
            cand.append((vals256[r], b, v))
            if len(cand) == VK + 16:
                break
        assert len(cand) >= VK, f"only {len(cand)} unique candidates for row {t}"
        # match jax.lax.top_k tie-break: equal values ordered by ascending flat idx
        cand.sort(key=lambda c: (-c[0], c[1] * VOCAB + c[2]))
        for k in range(VK):
            vals[t, k] = cand[k][0]
            vocab[t, k] = cand[k][2]
            beams[t, k] = 0 if step == 0 else cand[k][1]
    return vals, vocab, beams


def _run(lprobs: np.ndarray, scores: np.ndarray, step: int, trace: bool = False):
    from concourse.bass_utils import run_bass_kernel_spmd

    nc = _get_nc()
    in_maps = []
    for c in range(NCORES):
        shard = np.ascontiguousarray(lprobs[c * ROWS:(c + 1) * ROWS])
        scol, mcol, tcol, t8col, svec = _side_inputs(scores[c * ROWS:(c + 1) * ROWS], step)
        in_maps.append({"x": shard, "scol": scol, "mcol": mcol, "tcol": tcol,
                        "t8col": t8col, "svec": svec})
    res = run_bass_kernel_spmd(nc, in_maps, core_ids=list(range(NCORES)), trace=trace)
    return res


def kernel(lprobs, scores, step):
    lprobs = np.asarray(lprobs, dtype=np.float32)
    scores = np.asarray(scores, dtype=np.float32)
    step = int(step)

    res = _run(lprobs, scores, step)

    scores_buf = np.zeros((BSZ, VK), np.float32)
    indices_buf = np.zeros((BSZ, VK), np.int32)
    beams_buf = np.zeros((BSZ, VK), np.int32)
    for c in range(NCORES):
        o = res.results[c]
        v, vi, bi = _decode(o["out1"], o["out2"], step)
        rows = slice(c * ROWS, (c + 1) * ROWS)
        scores_buf[rows] = v
        indices_buf[rows] = vi
        beams_buf[rows] = bi
    return scores_buf, indices_buf, beams_buf


# revision 19
# speedup vs baseline: 34.0913x; 1.8904x over previous
"""Beam-search top-k (mask pad + add beam scores + top-16 over beam*vocab) on 8 trn2 cores.

Sharding: batch dim (64 rows) split across 8 cores, 8 rows/core, no cross-core comm.

Per-core device pipeline (Bass/Tile, pure DVE selection -- no gpsimd topk):
  tile [128, 25136] f32, partition p = (t*8+b)*2 + h  (t=batch row, b=beam, h=half)
     h=0 holds vocab [0, 25136); h=1 holds vocab [25121, 50257)
     (h=0's copy of the 15-element overlap is masked to -inf before reduction)
  1. 16 chunked DMAs, each all-128-partitions (the (t,b) dims merge into one
     stride-50257 dim so src APs stay 3-D); per-chunk segmented reduce_max over
     groups of 8 -> M [128, 3142] pipelines with the DMAs
  2. stage 1: per-partition top-16 groups of M via max8 / find_index8 /
     match_replace8 (HW resolves duplicate values to distinct positions in
     first-occurrence order, which matches jax.lax.top_k's lowest-index
     tie-break). Beam score is constant per partition, so selection doesn't
     need the bias.
  3. gather: each partition's 16 winning groups' raw 8 elements from DRAM via
     16 indirect DMAs (offset = rowbase[p] + col*8, f32-exact) -> Gc [128,128];
     add the per-partition beam score
  4. stage 3a: per-partition top-32 of Gc (4 max8 rounds); transpose the
     [128, 32] winners to [8, 512] (one token per partition); stage 3b:
     top-32 of each token with positions
  5. host decodes positions through the two index tables, drops the raw
     pad-token entries the gather may have pulled in, dedups the h-overlap
     duplicates, sorts equal values by flat index like jax, takes top-16.
"""

import sys

sys.path.insert(0, "/opt/trn_rl_repo")

import numpy as np

BSZ, BEAM, VOCAB, VK = 64, 8, 50257, 16
NCORES = 8
ROWS = BSZ // NCORES   # 8 tokens (batch rows) per core
F = 25136              # per-partition elems
CH0 = VOCAB - F        # 25121: h=1 partitions cover vocab [25121, 50257)
P = 128
G8 = 8                 # reduce group size
NG = F // G8           # 3142 groups per partition
NEL = ROWS * BEAM * VOCAB  # 3216448 elements in the per-core shard
NEG = float("-inf")
NEGBIG = -3.0e38       # finite stand-in for -inf in match_replace imm (json-safe)

_CACHE = {}


def _build():
    import concourse.bacc as bacc
    import concourse.bass as bass
    import concourse.mybir as mybir
    from concourse.bass_types import AP
    from concourse.tile import TileContext

    ALU = mybir.AluOpType

    nc = bacc.Bacc("TRN2", target_bir_lowering=False, debug=False)
    x = nc.dram_tensor("x", [ROWS, BEAM, VOCAB], mybir.dt.float32, kind="ExternalInput").ap()
    scol = nc.dram_tensor("scol", [P, 1], mybir.dt.float32, kind="ExternalInput").ap()
    mcol = nc.dram_tensor("mcol", [P, 1], mybir.dt.float32, kind="ExternalInput").ap()
    rbase = nc.dram_tensor("rbase", [P, 1], mybir.dt.float32, kind="ExternalInput").ap()
    o_i1 = nc.dram_tensor("o_i1", [P, 16], mybir.dt.uint32, kind="ExternalOutput").ap()
    o_i3a = nc.dram_tensor("o_i3a", [P, 32], mybir.dt.uint32, kind="ExternalOutput").ap()
    o_v = nc.dram_tensor("o_v", [ROWS, 32], mybir.dt.float32, kind="ExternalOutput").ap()
    o_i3b = nc.dram_tensor("o_i3b", [ROWS, 32], mybir.dt.uint32, kind="ExternalOutput").ap()

    with TileContext(nc) as tc:
        with tc.tile_pool(name="main", bufs=1) as pool:
            tile = pool.tile([P, F], mybir.dt.float32)
            M = pool.tile([P, NG], mybir.dt.float32)
            Mz = pool.tile([P, NG], mybir.dt.float32)
            sc = pool.tile([P, 1], mybir.dt.float32)
            mc = pool.tile([P, 1], mybir.dt.float32)
            rb = pool.tile([P, 1], mybir.dt.float32)
            A1 = pool.tile([P, 16], mybir.dt.float32)
            I1 = pool.tile([P, 16], mybir.dt.uint32)
            If = pool.tile([P, 16], mybir.dt.float32)
            Ef = pool.tile([P, 16], mybir.dt.float32)
            Eu = pool.tile([P, 16], mybir.dt.uint32)
            Gc = pool.tile([P, 128], mybir.dt.float32)
            Gz0 = pool.tile([P, 128], mybir.dt.float32)
            Gz1 = pool.tile([P, 128], mybir.dt.float32)
            A3 = pool.tile([P, 32], mybir.dt.float32)
            I3 = pool.tile([P, 32], mybir.dt.uint32)
            Gt = pool.tile([ROWS, 512], mybir.dt.float32)
            Gt0 = pool.tile([ROWS, 512], mybir.dt.float32)
            Gt1 = pool.tile([ROWS, 512], mybir.dt.float32)
            AB = pool.tile([ROWS, 32], mybir.dt.float32)
            IB = pool.tile([ROWS, 32], mybir.dt.uint32)

            nc.sync.dma_start(out=sc[:, :], in_=scol)
            nc.sync.dma_start(out=mc[:, :], in_=mcol)
            nc.sync.dma_start(out=rb[:, :], in_=rbase)

            # chunked loads, all 128 partitions per DMA; per-chunk reduce
            chunks = [(i * 1576, 1576) for i in range(15)] + [(15 * 1576, 1496)]
            first = True
            for o, ln in chunks:
                src = AP(
                    tensor=x.tensor, offset=o,
                    ap=[[VOCAB, ROWS * BEAM], [CH0, 2], [1, ln]],
                )
                nc.sync.dma_start(out=tile[:, o:o + ln], in_=src)
                if first:
                    # pad-token (vocab id 1) mask: -inf on h=0 partitions
                    nc.vector.tensor_scalar_add(tile[:, 1:2], tile[:, 1:2], mc[:, 0:1])
                    first = False
                if o + ln == F:
                    # kill h=0's copy of the overlap [25121, 25136) so stage 1
                    # never selects the duplicate (same -inf-on-even mask)
                    nc.vector.tensor_scalar_add(
                        tile[:, CH0:F], tile[:, CH0:F], mc[:, 0:1]
                    )
                t3 = tile[:, o:o + ln].rearrange("p (g w) -> p g w", w=G8)
                nc.vector.reduce_max(
                    out=M[:, o // G8:(o + ln) // G8], in_=t3, axis=mybir.AxisListType.X
                )

            # stage 1: per-partition top-16 groups (bias-free: score is
            # constant within a partition)
            nc.vector.max(out=A1[:, 0:8], in_=M[:, :])
            nc.vector.max_index(out=I1[:, 0:8], in_max=A1[:, 0:8], in_values=M[:, :])
            nc.vector.match_replace(
                out=Mz[:, :], in_to_replace=A1[:, 0:8], in_values=M[:, :],
                imm_value=NEGBIG,
            )
            nc.vector.max(out=A1[:, 8:16], in_=Mz[:, :])
            nc.vector.max_index(out=I1[:, 8:16], in_max=A1[:, 8:16], in_values=Mz[:, :])

            # gather offsets: E = rowbase[p] + col*8 (f32-exact, < 2^24)
            nc.vector.tensor_copy(out=If[:, :], in_=I1[:, :])
            nc.vector.tensor_scalar(
                out=Ef[:, :], in0=If[:, :], scalar1=float(G8), scalar2=rb[:, 0:1],
                op0=ALU.mult, op1=ALU.add,
            )
            nc.vector.tensor_copy(out=Eu[:, :], in_=Ef[:, :])

            xin = AP(tensor=x.tensor, offset=0, ap=[[1, NEL], [1, 1]])
            for r in range(16):
                nc.gpsimd.indirect_dma_start(
                    out=Gc[:, r * G8:(r + 1) * G8], out_offset=None, in_=xin,
                    in_offset=bass.IndirectOffsetOnAxis(ap=Eu[:, r:r + 1], axis=0),
                )
            nc.vector.tensor_scalar_add(Gc[:, :], Gc[:, :], sc[:, 0:1])

            # stage 3a: per-partition top-32 of the 128 gathered candidates
            srcs = [Gc, Gz0, Gz1, Gz0]
            for rd in range(4):
                s = srcs[rd]
                nc.vector.max(out=A3[:, rd * 8:rd * 8 + 8], in_=s[:, :])
                nc.vector.max_index(
                    out=I3[:, rd * 8:rd * 8 + 8], in_max=A3[:, rd * 8:rd * 8 + 8],
                    in_values=s[:, :],
                )
                if rd < 3:
                    nc.vector.match_replace(
                        out=srcs[rd + 1][:, :], in_to_replace=A3[:, rd * 8:rd * 8 + 8],
                        in_values=s[:, :], imm_value=NEGBIG,
                    )

            # transpose winners: token t's 16 partitions -> one partition row
            nc.sync.dma_start(out=Gt[:, :], in_=A3[:, :])

            # stage 3b: top-32 of each token's 512 candidates
            srcs = [Gt, Gt0, Gt1, Gt0]
            for rd in range(4):
                s = srcs[rd]
                nc.vector.max(out=AB[:, rd * 8:rd * 8 + 8], in_=s[:, :])
                nc.vector.max_index(
                    out=IB[:, rd * 8:rd * 8 + 8], in_max=AB[:, rd * 8:rd * 8 + 8],
                    in_values=s[:, :],
                )
                if rd < 3:
                    nc.vector.match_replace(
                        out=srcs[rd + 1][:, :], in_to_replace=AB[:, rd * 8:rd * 8 + 8],
                        in_values=s[:, :], imm_value=NEGBIG,
                    )

            nc.sync.dma_start(out=o_i1, in_=I1[:, :])
            nc.sync.dma_start(out=o_i3a, in_=I3[:, :])
            nc.sync.dma_start(out=o_v, in_=AB[:, :])
            nc.sync.dma_start(out=o_i3b, in_=IB[:, :])

    nc.compile()
    return nc


def _get_nc():
    if "nc" not in _CACHE:
        _CACHE["nc"] = _build()
    return _CACHE["nc"]


def _side_inputs(scores_shard: np.ndarray, step: int):
    scol = np.zeros((P, 1), np.float32)
    mcol = np.zeros((P, 1), np.float32)
    rbase = np.zeros((P, 1), np.float32)
    for t in range(ROWS):
        for b in range(BEAM):
            sv = (0.0 if b == 0 else NEG) if step == 0 else float(scores_shard[t, b])
            for h in range(2):
                p = t * 16 + b * 2 + h
                scol[p, 0] = sv
                if h == 0:
                    mcol[p, 0] = NEG
                rbase[p, 0] = float((t * BEAM + b) * VOCAB + h * CH0)
    return scol, mcol, rbase


def _decode(o_i1, o_i3a, o_v, o_i3b, step: int):
    vals = np.zeros((ROWS, VK), np.float32)
    vocab = np.zeros((ROWS, VK), np.int32)
    beams = np.zeros((ROWS, VK), np.int32)
    for t in range(ROWS):
        cand = []  # (val, beam, vocab)
        seen = set()
        vrow = o_v[t]
        exhausted = True  # capture covered everything down to -inf padding
        for s_ in range(32):
            val = vrow[s_]
            if val < -1e37 or not np.isfinite(val):
                break
            pos_b = int(o_i3b[t, s_])          # in [0, 512)
            q, u = divmod(pos_b, 32)
            pos_a = int(o_i3a[t * 16 + q, u])  # in [0, 128)
            r, e = divmod(pos_a, G8)
            col = int(o_i1[t * 16 + q, r])     # group col in [0, NG)
            b, h = divmod(q, 2)
            v = h * CH0 + col * G8 + e
            if v == 1:
                continue  # pad token pulled in raw by the gather
            key = (b, v)
            if key in seen:
                continue  # h-overlap duplicate
            seen.add(key)
            cand.append((val, b, v))
        else:
            exhausted = False  # all 32 captured slots were live candidates
        assert len(cand) >= VK, f"only {len(cand)} unique candidates for row {t}"
        cand.sort(key=lambda c: (-c[0], c[1] * VOCAB + c[2]))
        # guard: if the 16th value ties with the last captured rank and the
        # capture wasn't exhaustive, a tie cluster might extend past the
        # top-32 window -- refuse rather than be silently wrong
        assert exhausted or cand[VK - 1][0] > vrow[31], (
            f"tie cluster may straddle the top-32 capture for row {t}"
        )
        for k in range(VK):
            vals[t, k] = cand[k][0]
            vocab[t, k] = cand[k][2]
            beams[t, k] = 0 if step == 0 else cand[k][1]
    return vals, vocab, beams


def _run(lprobs: np.ndarray, scores: np.ndarray, step: int, trace: bool = False):
    from concourse.bass_utils import run_bass_kernel_spmd

    nc = _get_nc()
    in_maps = []
    for c in range(NCORES):
        shard = np.ascontiguousarray(lprobs[c * ROWS:(c + 1) * ROWS])
        scol, mcol, rbase = _side_inputs(scores[c * ROWS:(c + 1) * ROWS], step)
        in_maps.append({"x": shard, "scol": scol, "mcol": mcol, "rbase": rbase})
    res = run_bass_kernel_spmd(nc, in_maps, core_ids=list(range(NCORES)), trace=trace)
    return res


def kernel(lprobs, scores, step):
    lprobs = np.asarray(lprobs, dtype=np.float32)
    scores = np.asarray(scores, dtype=np.float32)
    step = int(step)

    res = _run(lprobs, scores, step)

    scores_buf = np.zeros((BSZ, VK), np.float32)
    indices_buf = np.zeros((BSZ, VK), np.int32)
    beams_buf = np.zeros((BSZ, VK), np.int32)
    for c in range(NCORES):
        o = res.results[c]
        v, vi, bi = _decode(o["o_i1"], o["o_i3a"], o["o_v"], o["o_i3b"], step)
        rows = slice(c * ROWS, (c + 1) * ROWS)
        scores_buf[rows] = v
        indices_buf[rows] = vi
        beams_buf[rows] = bi
    return scores_buf, indices_buf, beams_buf


# revision 20
# speedup vs baseline: 39.3927x; 1.1555x over previous
"""Beam-search top-k (mask pad + add beam scores + top-16 over beam*vocab) on 8 trn2 cores.

Sharding: batch dim (64 rows) split across 8 cores, 8 rows/core, no cross-core comm.

Per-core device pipeline (Bass/Tile, pure DVE selection -- no gpsimd topk):
  tile [128, 25136] f32, partition p = (t*8+b)*2 + h  (t=batch row, b=beam, h=half)
     h=0 holds vocab [0, 25136); h=1 holds vocab [25121, 50257)
     (h=0's copy of the 15-element overlap is masked to -inf before reduction)
  1. 16 chunked DMAs, each all-128-partitions (the (t,b) dims merge into one
     stride-50257 dim so src APs stay 3-D); per-chunk segmented reduce_max over
     groups of 16 -> M [128, 1571] pipelines with the DMAs
  2. stage 1: per-partition top-16 groups of M via max8 / find_index8 /
     match_replace8 (HW resolves duplicate values to distinct positions in
     first-occurrence order, which matches jax.lax.top_k's lowest-index
     tie-break). Beam score is constant per partition, so selection doesn't
     need the bias.
  3. gather: each partition's 16 winning groups' raw 16 elements from DRAM via
     2 batches of 8 indirect DMAs (offset = rowbase[p] + col*16, f32-exact);
     batch 1 overlaps stage 1's second extraction round. Add the per-partition
     beam score -> Gc [128, 256]
  4. stage 3a: per-partition top-32 of Gc (4 max8 rounds); transpose the
     [128, 32] winners to [8, 512] (one token per partition); stage 3b:
     top-32 of each token with positions
  5. host decodes positions through the two index tables, drops the raw
     pad-token entries the gather may have pulled in, dedups the h-overlap
     duplicates, sorts equal values by flat index like jax, takes top-16.
"""

import sys

sys.path.insert(0, "/opt/trn_rl_repo")

import numpy as np

BSZ, BEAM, VOCAB, VK = 64, 8, 50257, 16
NCORES = 8
ROWS = BSZ // NCORES   # 8 tokens (batch rows) per core
F = 25136              # per-partition elems
CH0 = VOCAB - F        # 25121: h=1 partitions cover vocab [25121, 50257)
P = 128
GW = 16                # reduce group width
NG = F // GW           # 1571 groups per partition
NEL = ROWS * BEAM * VOCAB  # 3216448 elements in the per-core shard
NEG = float("-inf")
NEGBIG = -3.0e38       # finite stand-in for -inf in match_replace imm (json-safe)

_CACHE = {}


def _build():
    import concourse.bacc as bacc
    import concourse.bass as bass
    import concourse.mybir as mybir
    from concourse.bass_types import AP
    from concourse.tile import TileContext

    ALU = mybir.AluOpType

    nc = bacc.Bacc("TRN2", target_bir_lowering=False, debug=False)
    x = nc.dram_tensor("x", [ROWS, BEAM, VOCAB], mybir.dt.float32, kind="ExternalInput").ap()
    scol = nc.dram_tensor("scol", [P, 1], mybir.dt.float32, kind="ExternalInput").ap()
    mcol = nc.dram_tensor("mcol", [P, 1], mybir.dt.float32, kind="ExternalInput").ap()
    rbase = nc.dram_tensor("rbase", [P, 1], mybir.dt.float32, kind="ExternalInput").ap()
    o_i1 = nc.dram_tensor("o_i1", [P, 16], mybir.dt.uint32, kind="ExternalOutput").ap()
    o_i3a = nc.dram_tensor("o_i3a", [P, 32], mybir.dt.uint32, kind="ExternalOutput").ap()
    o_v = nc.dram_tensor("o_v", [ROWS, 32], mybir.dt.float32, kind="ExternalOutput").ap()
    o_i3b = nc.dram_tensor("o_i3b", [ROWS, 32], mybir.dt.uint32, kind="ExternalOutput").ap()

    with TileContext(nc) as tc:
        with tc.tile_pool(name="main", bufs=1) as pool:
            tile = pool.tile([P, F], mybir.dt.float32)
            M = pool.tile([P, NG], mybir.dt.float32)
            Mz = pool.tile([P, NG], mybir.dt.float32)
            sc = pool.tile([P, 1], mybir.dt.float32)
            mc = pool.tile([P, 1], mybir.dt.float32)
            rb = pool.tile([P, 1], mybir.dt.float32)
            A1 = pool.tile([P, 16], mybir.dt.float32)
            I1 = pool.tile([P, 16], mybir.dt.uint32)
            If = pool.tile([P, 16], mybir.dt.float32)
            Ef = pool.tile([P, 16], mybir.dt.float32)
            Eu = pool.tile([P, 16], mybir.dt.uint32)
            Gc = pool.tile([P, 16 * GW], mybir.dt.float32)
            Gz0 = pool.tile([P, 16 * GW], mybir.dt.float32)
            Gz1 = pool.tile([P, 16 * GW], mybir.dt.float32)
            A3 = pool.tile([P, 32], mybir.dt.float32)
            I3 = pool.tile([P, 32], mybir.dt.uint32)
            Gt = pool.tile([ROWS, 512], mybir.dt.float32)
            Gt0 = pool.tile([ROWS, 512], mybir.dt.float32)
            Gt1 = pool.tile([ROWS, 512], mybir.dt.float32)
            AB = pool.tile([ROWS, 32], mybir.dt.float32)
            IB = pool.tile([ROWS, 32], mybir.dt.uint32)

            nc.sync.dma_start(out=sc[:, :], in_=scol)
            nc.sync.dma_start(out=mc[:, :], in_=mcol)
            nc.sync.dma_start(out=rb[:, :], in_=rbase)

            # chunked loads, all 128 partitions per DMA; per-chunk reduce
            chunks = [(i * 1568, 1568) for i in range(15)] + [(15 * 1568, 1616)]
            first = True
            for o, ln in chunks:
                src = AP(
                    tensor=x.tensor, offset=o,
                    ap=[[VOCAB, ROWS * BEAM], [CH0, 2], [1, ln]],
                )
                nc.sync.dma_start(out=tile[:, o:o + ln], in_=src)
                if first:
                    # pad-token (vocab id 1) mask: -inf on h=0 partitions
                    nc.vector.tensor_scalar_add(tile[:, 1:2], tile[:, 1:2], mc[:, 0:1])
                    first = False
                if o + ln == F:
                    # kill h=0's copy of the overlap [25121, 25136) so stage 1
                    # never selects the duplicate (same -inf-on-even mask)
                    nc.vector.tensor_scalar_add(
                        tile[:, CH0:F], tile[:, CH0:F], mc[:, 0:1]
                    )
                t3 = tile[:, o:o + ln].rearrange("p (g w) -> p g w", w=GW)
                nc.vector.reduce_max(
                    out=M[:, o // GW:(o + ln) // GW], in_=t3, axis=mybir.AxisListType.X
                )

            xin = AP(tensor=x.tensor, offset=0, ap=[[1, NEL], [1, 1]])

            def emit_gather_batch(lo, hi):
                # offsets E = rowbase[p] + col*GW (f32-exact, < 2^24)
                nc.vector.tensor_copy(out=If[:, lo:hi], in_=I1[:, lo:hi])
                nc.vector.tensor_scalar(
                    out=Ef[:, lo:hi], in0=If[:, lo:hi], scalar1=float(GW),
                    scalar2=rb[:, 0:1], op0=ALU.mult, op1=ALU.add,
                )
                nc.vector.tensor_copy(out=Eu[:, lo:hi], in_=Ef[:, lo:hi])
                for r in range(lo, hi):
                    nc.gpsimd.indirect_dma_start(
                        out=Gc[:, r * GW:(r + 1) * GW], out_offset=None, in_=xin,
                        in_offset=bass.IndirectOffsetOnAxis(ap=Eu[:, r:r + 1], axis=0),
                    )

            # stage 1: per-partition top-16 groups (bias-free: score is
            # constant within a partition); gathers interleave with round 2
            nc.vector.max(out=A1[:, 0:8], in_=M[:, :])
            nc.vector.max_index(out=I1[:, 0:8], in_max=A1[:, 0:8], in_values=M[:, :])
            emit_gather_batch(0, 8)
            nc.vector.match_replace(
                out=Mz[:, :], in_to_replace=A1[:, 0:8], in_values=M[:, :],
                imm_value=NEGBIG,
            )
            nc.vector.max(out=A1[:, 8:16], in_=Mz[:, :])
            nc.vector.max_index(out=I1[:, 8:16], in_max=A1[:, 8:16], in_values=Mz[:, :])
            emit_gather_batch(8, 16)

            nc.sync.dma_start(out=o_i1, in_=I1[:, :])

            nc.vector.tensor_scalar_add(Gc[:, :], Gc[:, :], sc[:, 0:1])

            # stage 3a: per-partition top-32 of the 256 gathered candidates
            srcs = [Gc, Gz0, Gz1, Gz0]
            for rd in range(4):
                s = srcs[rd]
                nc.vector.max(out=A3[:, rd * 8:rd * 8 + 8], in_=s[:, :])
                nc.vector.max_index(
                    out=I3[:, rd * 8:rd * 8 + 8], in_max=A3[:, rd * 8:rd * 8 + 8],
                    in_values=s[:, :],
                )
                if rd < 3:
                    nc.vector.match_replace(
                        out=srcs[rd + 1][:, :], in_to_replace=A3[:, rd * 8:rd * 8 + 8],
                        in_values=s[:, :], imm_value=NEGBIG,
                    )
            nc.sync.dma_start(out=o_i3a, in_=I3[:, :])

            # transpose winners: token t's 16 partitions -> one partition row
            nc.sync.dma_start(out=Gt[:, :], in_=A3[:, :])

            # stage 3b: top-32 of each token's 512 candidates
            srcs = [Gt, Gt0, Gt1, Gt0]
            for rd in range(4):
                s = srcs[rd]
                nc.vector.max(out=AB[:, rd * 8:rd * 8 + 8], in_=s[:, :])
                nc.vector.max_index(
                    out=IB[:, rd * 8:rd * 8 + 8], in_max=AB[:, rd * 8:rd * 8 + 8],
                    in_values=s[:, :],
                )
                if rd < 3:
                    nc.vector.match_replace(
                        out=srcs[rd + 1][:, :], in_to_replace=AB[:, rd * 8:rd * 8 + 8],
                        in_values=s[:, :], imm_value=NEGBIG,
                    )

            nc.sync.dma_start(out=o_v, in_=AB[:, :])
            nc.sync.dma_start(out=o_i3b, in_=IB[:, :])

    nc.compile()
    return nc


def _get_nc():
    if "nc" not in _CACHE:
        _CACHE["nc"] = _build()
    return _CACHE["nc"]


def _side_inputs(scores_shard: np.ndarray, step: int):
    scol = np.zeros((P, 1), np.float32)
    mcol = np.zeros((P, 1), np.float32)
    rbase = np.zeros((P, 1), np.float32)
    for t in range(ROWS):
        for b in range(BEAM):
            sv = (0.0 if b == 0 else NEG) if step == 0 else float(scores_shard[t, b])
            for h in range(2):
                p = t * 16 + b * 2 + h
                scol[p, 0] = sv
                if h == 0:
                    mcol[p, 0] = NEG
                rbase[p, 0] = float((t * BEAM + b) * VOCAB + h * CH0)
    return scol, mcol, rbase


def _decode(o_i1, o_i3a, o_v, o_i3b, step: int):
    vals = np.zeros((ROWS, VK), np.float32)
    vocab = np.zeros((ROWS, VK), np.int32)
    beams = np.zeros((ROWS, VK), np.int32)
    for t in range(ROWS):
        cand = []  # (val, beam, vocab)
        seen = set()
        vrow = o_v[t]
        exhausted = True  # capture covered everything down to the padding
        for s_ in range(32):
            val = vrow[s_]
            if val < -1e37 or not np.isfinite(val):
                break
            pos_b = int(o_i3b[t, s_])          # in [0, 512)
            q, u = divmod(pos_b, 32)
            pos_a = int(o_i3a[t * 16 + q, u])  # in [0, 256)
            r, e = divmod(pos_a, GW)
            col = int(o_i1[t * 16 + q, r])     # group col in [0, NG)
            b, h = divmod(q, 2)
            v = h * CH0 + col * GW + e
            if v == 1:
                continue  # pad token pulled in raw by the gather
            key = (b, v)
            if key in seen:
                continue  # h-overlap duplicate
            seen.add(key)
            cand.append((val, b, v))
        else:
            exhausted = False  # all 32 captured slots were live candidates
        assert len(cand) >= VK, f"only {len(cand)} unique candidates for row {t}"
        cand.sort(key=lambda c: (-c[0], c[1] * VOCAB + c[2]))
        # guard: if the 16th value ties with the last captured rank and the
        # capture wasn't exhaustive, a tie cluster might extend past the
        # top-32 window -- refuse rather than be silently wrong
        assert exhausted or cand[VK - 1][0] > vrow[31], (
            f"tie cluster may straddle the top-32 capture for row {t}"
        )
        for k in range(VK):
            vals[t, k] = cand[k][0]
            vocab[t, k] = cand[k][2]
            beams[t, k] = 0 if step == 0 else cand[k][1]
    return vals, vocab, beams


def _run(lprobs: np.ndarray, scores: np.ndarray, step: int, trace: bool = False):
    from concourse.bass_utils import run_bass_kernel_spmd

    nc = _get_nc()
    in_maps = []
    for c in range(NCORES):
        shard = np.ascontiguousarray(lprobs[c * ROWS:(c + 1) * ROWS])
        scol, mcol, rbase = _side_inputs(scores[c * ROWS:(c + 1) * ROWS], step)
        in_maps.append({"x": shard, "scol": scol, "mcol": mcol, "rbase": rbase})
    res = run_bass_kernel_spmd(nc, in_maps, core_ids=list(range(NCORES)), trace=trace)
    return res


def kernel(lprobs, scores, step):
    lprobs = np.asarray(lprobs, dtype=np.float32)
    scores = np.asarray(scores, dtype=np.float32)
    step = int(step)

    res = _run(lprobs, scores, step)

    scores_buf = np.zeros((BSZ, VK), np.float32)
    indices_buf = np.zeros((BSZ, VK), np.int32)
    beams_buf = np.zeros((BSZ, VK), np.int32)
    for c in range(NCORES):
        o = res.results[c]
        v, vi, bi = _decode(o["o_i1"], o["o_i3a"], o["o_v"], o["o_i3b"], step)
        rows = slice(c * ROWS, (c + 1) * ROWS)
        scores_buf[rows] = v
        indices_buf[rows] = vi
        beams_buf[rows] = bi
    return scores_buf, indices_buf, beams_buf
